# revision 7
# baseline (speedup 1.0000x reference)
"""Bass/Trainium2 kernel for nn_Attn_1185410973711 (additive attention scores).

Computation (reference, fp32):
    W_s = W_attn[:, :H]; W_e = W_attn[:, H:]
    energy  = tanh(output @ W_s.T [:,None,:] + einsum('bse,he->bsh', enc, W_e) + b_attn)
    scores  = einsum('bsh,h->bs', energy, v) - 1000*(mask==0)
    out     = softmax(scores, axis=-1)           # [B, 1, S]

Strategy: data-parallel over batch B=32 across 8 NeuronCores (4 batches per
core); W_attn/b_attn/v replicated.

Mask compaction: positions with encoder_mask==0 receive a -1000 penalty, and
exp(-1000-ish) underflows to exactly 0.0 in fp32, so masked positions
contribute nothing to the softmax numerator or denominator. The host gathers
only the unmasked encoder columns per batch (~50% with this input
distribution), pads each batch to a whole number of 128-column tiles (padded
columns carry the -1000 penalty themselves, so they also produce exact
zeros), runs the kernel on the compacted sequence, and scatters results back
into a zero [B, 1, S] output. This nearly halves the dominant matmul.

Slot-sorted assignment: the SPMD program is shared by all 8 cores, so the
per-slot tile count must only match across cores, not across the 4 batch
slots. Batches are sorted by unmasked count and rank-grouped into slots, so
short batches don't get padded to the global worst case.

The compacted encoder block (bf16, pre-transposed to [e, s] on the host) is
DMA'd fully into SBUF in the prologue; the steady-state loop runs with zero
input DMA. The enc_proj matmul keeps enc tiles stationary / W_e moving so
PSUM lands as [s_part, h_free]; the v-dot leaves the PE (DVE multiply +
scalar-engine accumulate along the free axis). The softmax tail is a single
drain: one mask-add over all tiles, ONE PE transpose of the whole
[128, sum(NT)] score block, one exp with fused row-sums, and per-batch
totals/broadcasts via two small segment-indicator matmuls, so no PE stall
ever interrupts the matmul stream.
"""

import contextlib
import math

import numpy as np

B, S, H = 32, 2048, 512
E2 = 2 * H            # 1024, encoder feature dim
N_CORES = 8
BPC = B // N_CORES    # 4 batches per core
NK = E2 // 128        # 8 contraction tiles


def _split_drain_context(nc):
    """TileContext subclass working around a walrus limit in this build: the
    kernel-tail drain rejects instructions carrying more than one semaphore
    wait. See enforce_wait_limit()."""
    import concourse.tile as tile
    from concourse.vector_clock import ScopedClock

    class TileContextSplitDrain(tile.TileContext):
        def _drain_and_barrier(self, tick_clock, wait_clock):
            probe = self.nc.sync.nop(nofuse=True, hint="tail_wait_probe")
            wait_clock.add_sem_waits(
                probe.ins, ScopedClock({None: tick_clock.global_clock})
            )
            si = probe.ins.sync_info
            waits = list(si.on_wait or []) if si is not None else []
            if si is not None:
                si.on_wait.clear()
            by_name = {h.name: h for h in self.sems.allocated().values()}
            for w in waits:
                h = by_name.get(w.ant_name)
                assert h is not None, f"missing semaphore handle for {w.ant_name}"
                self.nc.sync.wait_ge(h, w.wait_value)
            self.nc.sync.drain()
            self.nc.all_engine_barrier()
            popped = self.nc._tile_sem_poison_stack.pop()
            assert popped is self._sem_poison
            self.nc.clear_and_free_semaphores(list(self.sems.allocated().values()))
            self.nc.all_engine_barrier()

    return TileContextSplitDrain(nc)


def enforce_wait_limit(nc, limit=1):
    """Hoist excess semaphore waits onto inserted same-engine event-sem wait
    instructions placed immediately before the over-budget instruction.
    In-order engine execution makes an earlier wait strictly conservative,
    so this is always sound. Several opcodes in this walrus build (notably
    self-loading fp32 matmuls and Drain) reject multi-wait encodings."""
    import copy

    template = None
    for fn in nc.m.functions:
        for bb in fn.blocks:
            for ins in bb.instructions:
                if type(ins).__name__ == "InstEventSemaphore":
                    si = ins.sync_info
                    if si and si.on_wait and len(si.on_wait) == 1:
                        template = ins
                        break
            if template:
                break
        if template:
            break

    n_new = 0
    for fn in nc.m.functions:
        for bb in fn.blocks:
            il = bb.instructions
            new_il = []
            changed = False
            for ins in il:
                si = ins.sync_info
                waits = list(si.on_wait) if si and si.on_wait else []
                if len(waits) > limit and type(ins).__name__ != "InstEventSemaphore":
                    assert template is not None, "no event-sem template found"
                    for w in waits[limit:]:
                        c = copy.deepcopy(template)
                        n_new += 1
                        c.name = f"I-waitfix-{n_new}"
                        c.engine = ins.engine
                        csi = c.sync_info
                        csi.on_wait.clear()
                        csi.on_wait.append(w)
                        csi.on_update.clear()
                        new_il.append(c)
                    si.on_wait.clear()
                    for w in waits[:limit]:
                        si.on_wait.append(w)
                    changed = True
                new_il.append(ins)
            if changed:
                il[:] = new_il
    return n_new


def _plan(encoder_mask):
    """Slot-sorted batch assignment. Returns (order, counts, ntj) where
    order[8*j + c] is the original batch index handled by core c, slot j;
    ntj[j] is the per-slot compacted tile count (max over cores)."""
    counts = np.asarray(encoder_mask != 0).sum(axis=1).astype(int)
    order = np.argsort(-counts, kind="stable")
    ntj = []
    for j in range(BPC):
        grp = counts[order[j * N_CORES:(j + 1) * N_CORES]]
        ntj.append(max(1, int(math.ceil(int(grp.max()) / 128))))
    return order, counts, ntj


def build_nc(reps=1, ntj=(9, 9, 8, 8)):
    """Build the per-core Bass program: per-slot compacted sequences of
    ntj[j]*128 columns concatenated along s. reps>1 wraps the steady-state
    body in a For_i loop re-running the identical computation (for timing)."""
    import concourse.bass as bass
    from concourse import mybir

    f32 = mybir.dt.float32
    bf16 = mybir.dt.bfloat16
    Tanh = mybir.ActivationFunctionType.Tanh
    Exp = mybir.ActivationFunctionType.Exp
    Ident = mybir.ActivationFunctionType.Identity

    ntj = list(ntj)
    NTS = sum(ntj)                       # total tiles across slots
    off = [sum(ntj[:j]) for j in range(BPC)]
    slot_of = []
    for j in range(BPC):
        slot_of += [j] * ntj[j]
    SC = NTS * 128                       # flat compacted columns per core

    nc = bass.Bass("TRN2", target_bir_lowering=False, debug=False)

    encC_d = nc.dram_tensor("encC", [NK, 128, SC], bf16, kind="ExternalInput")
    weT_d = nc.dram_tensor("weT", [2 * H, H], bf16, kind="ExternalInput")
    wsT_d = nc.dram_tensor("wsT", [H, H], bf16, kind="ExternalInput")
    outB_d = nc.dram_tensor("outB", [BPC, 4, 128, 128], bf16, kind="ExternalInput")
    bAR_d = nc.dram_tensor("bAR", [128, H], f32, kind="ExternalInput")
    vR_d = nc.dram_tensor("vR", [128, H], bf16, kind="ExternalInput")
    mkC_d = nc.dram_tensor("mkC", [NTS, 128], f32, kind="ExternalInput")
    segA_d = nc.dram_tensor("segA", [NTS, BPC], f32, kind="ExternalInput")
    segB_d = nc.dram_tensor("segB", [BPC, NTS], f32, kind="ExternalInput")
    eye_d = nc.dram_tensor("eye", [128, 128], f32, kind="ExternalInput")
    out_d = nc.dram_tensor("out", [NTS, 128], f32, kind="ExternalOutput")

    tc = _split_drain_context(nc)
    with tc:
        with contextlib.ExitStack() as ctx:
            const = ctx.enter_context(tc.tile_pool(name="const", bufs=1))
            prep = ctx.enter_context(tc.tile_pool(name="prep", bufs=6))
            enrg = ctx.enter_context(tc.tile_pool(name="enrg", bufs=6))
            scrp = ctx.enter_context(tc.tile_pool(name="scrp", bufs=4))
            rowp = ctx.enter_context(tc.tile_pool(name="rowp", bufs=1))
            pe_p = ctx.enter_context(tc.tile_pool(name="pe_p", bufs=7, space="PSUM"))
            ms_p = ctx.enter_context(tc.tile_pool(name="ms_p", bufs=1, space="PSUM"))

            enc_sb = const.tile([128, NK, SC], bf16)
            we_sb = const.tile([128, NK, H], bf16)        # W_e.T tiles [e,k,h]
            ws_sb = const.tile([128, H // 128, H], bf16)  # W_s.T tiles
            ob_sb = const.tile([128, BPC, H // 128, 128], bf16)  # output bcast
            bAR_sb = const.tile([128, H], f32)
            vR_sb = const.tile([128, H], bf16)
            mk_sb = const.tile([128, NTS], f32)
            segA_sb = const.tile([NTS, BPC], f32)   # 1 in rows of slot j, col j
            segB_sb = const.tile([BPC, NTS], f32)   # transpose of segA
            eye_sb = const.tile([128, 128], f32)

            nc.sync.dma_start(we_sb[:], weT_d.ap().rearrange("(k p) h -> p k h", p=128))
            nc.sync.dma_start(
                ws_sb[:], wsT_d.ap().rearrange("(k p) h -> p k h", p=128)
            )
            nc.sync.dma_start(
                ob_sb[:], outB_d.ap().rearrange("b k p m -> p b k m")
            )
            nc.sync.dma_start(bAR_sb[:], bAR_d.ap()[:])
            nc.sync.dma_start(vR_sb[:], vR_d.ap()[:])
            nc.sync.dma_start(mk_sb[:], mkC_d.ap().rearrange("t p -> p t"))
            nc.sync.dma_start(segA_sb[:], segA_d.ap()[:])
            nc.sync.dma_start(segB_sb[:], segB_d.ap()[:])
            nc.sync.dma_start(eye_sb[:], eye_d.ap()[:])
            # compacted encoder block, resident for the whole kernel
            for k in range(NK):
                nc.sync.dma_start(
                    enc_sb[:, k, :], encC_d.ap()[k]
                )

            # ---- c_rep[j] = broadcast(output[j] @ W_s.T + b_attn) ---------
            # outB is output[j] replicated along M on the host, so the state
            # matmul directly yields the row-broadcast [128, H] result; also
            # serves as the PE warm-up burst during the enc DMA.
            c_rep = const.tile([128, BPC, H], f32)
            for j in range(BPC):
                pc = ms_p.tile([128, H], f32, tag="misc", name=f"pc{j}")
                for k in range(H // 128):
                    nc.tensor.matmul(
                        pc[:],
                        ob_sb[:, j, k, :],
                        ws_sb[:, k, :],
                        start=(k == 0),
                        stop=(k == H // 128 - 1),
                    )
                nc.vector.tensor_add(c_rep[:, j, :], pc[:], bAR_sb[:])

            def body(_iv=None):
                sccols = rowp.tile([128, NTS], f32, tag="sccols")
                expv = rowp.tile([NTS, 128], f32, tag="expv")
                accT = rowp.tile([NTS, 1], f32, tag="accT")
                outv = rowp.tile([NTS, 128], f32, tag="outv")

                for t in range(NTS):
                    j = slot_of[t]
                    ps = pe_p.tile([128, H], f32, tag="pe")
                    for k in range(NK):
                        nc.tensor.matmul(
                            ps[:],
                            enc_sb[:, k, t * 128:(t + 1) * 128],
                            we_sb[:, k, :],
                            start=(k == 0),
                            stop=(k == NK - 1),
                        )
                    pre = prep.tile([128, H], f32, tag="pre")
                    nc.vector.tensor_add(pre[:], ps[:], c_rep[:, j, :])
                    en = enrg.tile([128, H], bf16, tag="en")
                    nc.scalar.activation(en[:], pre[:], Tanh)
                    scr = scrp.tile([128, H], bf16, tag="scr")
                    nc.vector.tensor_mul(scr[:], en[:], vR_sb[:])
                    dmp = scrp.tile([128, H], bf16, tag="dmp")
                    # bias = mask_penalty/H folded into the H-element accum:
                    # accum = sum_h(scr_h + mk/H) = score + mask_penalty
                    nc.scalar.activation(
                        dmp[:], scr[:], Ident,
                        bias=mk_sb[:, t:t + 1],
                        accum_out=sccols[:, t:t + 1],
                    )

                # ---- softmax drain: no mid-stream PE stalls ---------------
                tp = ms_p.tile([NTS, 128], f32, tag="misc", name="tp")
                nc.tensor.transpose(tp[:], sccols[:], eye_sb[:])
                nc.scalar.activation(
                    expv[:], tp[:], Exp, accum_out=accT[:],
                )
                tot = ms_p.tile([BPC, 1], f32, tag="misc", name="tot")
                nc.tensor.matmul(tot[:], segA_sb[:], accT[:], start=True, stop=True)
                rec = rowp.tile([BPC, 1], f32, tag="rec", name="rec")
                nc.vector.reciprocal(rec[:], tot[:])
                rb = ms_p.tile([NTS, 1], f32, tag="misc", name="rb")
                nc.tensor.matmul(rb[:], segB_sb[:], rec[:], start=True, stop=True)
                rec_sb = rowp.tile([NTS, 1], f32, tag="rec_sb", name="recs")
                nc.vector.tensor_copy(rec_sb[:], rb[:])
                nc.vector.tensor_scalar_mul(outv[:], expv[:], rec_sb[:])
                nc.sync.dma_start(out_d.ap()[:], outv[:])

            if reps == 1:
                body()
            else:
                from concourse import mybir as _mb

                with tc.For_i(
                    0, reps, 1,
                    hint_engines=(
                        _mb.EngineType.PE, _mb.EngineType.Activation,
                        _mb.EngineType.SP, _mb.EngineType.DVE,
                    ),
                ):
                    body()

    enforce_wait_limit(nc)
    return nc


def _shard_inputs(output, encoder_outputs, encoder_mask, W_attn, b_attn, v):
    """Returns (in_maps, meta) where meta = (order, counts, ntj) describes
    the slot-sorted batch assignment for the output scatter."""
    import ml_dtypes

    order, counts, ntj = _plan(encoder_mask)
    NTS = sum(ntj)
    off = [sum(ntj[:j]) for j in range(BPC)]
    SC = NTS * 128

    wT32 = np.ascontiguousarray(W_attn.T.astype(np.float32))        # [1536, 512]
    weT = wT32[H:].astype(ml_dtypes.bfloat16)                       # [1024, 512]
    wsT = wT32[:H].astype(ml_dtypes.bfloat16)                       # [512, 512]
    eye = np.eye(128, dtype=np.float32)
    bAR = np.broadcast_to(b_attn.astype(np.float32), (128, H)).copy()
    vR = np.broadcast_to(
        v.astype(np.float32).astype(ml_dtypes.bfloat16), (128, H)
    ).copy()
    segA = np.zeros((NTS, BPC), dtype=np.float32)
    for j in range(BPC):
        segA[off[j]:off[j] + ntj[j], j] = 1.0
    segB = np.ascontiguousarray(segA.T)

    idx_list = [np.nonzero(encoder_mask[b] != 0)[0] for b in range(B)]

    in_maps = []
    for c in range(N_CORES):
        encC = np.zeros((SC, E2), dtype=ml_dtypes.bfloat16)
        # penalty is pre-divided by H: the kernel folds it into the
        # H-element accumulate as a per-partition bias (H adds of mk/H).
        # -1000/512 = -1.953125 is exact in fp32.
        mkC = np.full((SC,), -1000.0 / H, dtype=np.float32)
        outs = np.empty((BPC, H), dtype=np.float32)
        for j in range(BPC):
            gb = int(order[j * N_CORES + c])
            ix = idx_list[gb]
            s0 = off[j] * 128
            encC[s0:s0 + len(ix)] = encoder_outputs[gb, ix].astype(
                ml_dtypes.bfloat16
            )
            mkC[s0:s0 + len(ix)] = 0.0
            outs[j] = output[gb].astype(np.float32)
        # transpose to [e, s] layout for contraction-ready DMA
        encT = np.ascontiguousarray(encC.T).reshape(NK, 128, SC)
        outB = np.broadcast_to(
            outs.astype(ml_dtypes.bfloat16).reshape(BPC, 4, 128, 1),
            (BPC, 4, 128, 128),
        ).copy()
        in_maps.append({
            "encC": encT, "weT": weT, "wsT": wsT, "outB": outB,
            "bAR": bAR, "vR": vR, "mkC": mkC.reshape(NTS, 128),
            "segA": segA, "segB": segB, "eye": eye,
        })
    return in_maps, (order, counts, ntj, idx_list)


def kernel(output, encoder_outputs, encoder_mask, W_attn, b_attn, v):
    from concourse.bass_utils import run_bass_kernel_spmd

    output = np.asarray(output)
    encoder_outputs = np.asarray(encoder_outputs)
    encoder_mask = np.asarray(encoder_mask)
    W_attn = np.asarray(W_attn)
    b_attn = np.asarray(b_attn)
    v = np.asarray(v)

    in_maps, (order, counts, ntj, idx_list) = _shard_inputs(
        output, encoder_outputs, encoder_mask, W_attn, b_attn, v
    )
    off = [sum(ntj[:j]) for j in range(BPC)]
    nc = build_nc(ntj=ntj)
    res = run_bass_kernel_spmd(nc, in_maps, core_ids=list(range(N_CORES)))
    full = np.zeros((B, S), dtype=np.float32)
    for j in range(BPC):
        for c in range(N_CORES):
            gb = int(order[j * N_CORES + c])
            cnt, ix = int(counts[gb]), idx_list[gb]
            if cnt == 0:
                # all positions masked: softmax over uniform -1000 scores
                full[gb] = 1.0 / S
                continue
            probs = res.results[c]["out"][off[j]:off[j] + ntj[j]].reshape(-1)
            full[gb, ix] = probs[:cnt]
    return full.reshape(B, 1, S)
